# revision 8
# baseline (speedup 1.0000x reference)
"""CenterLoss Trainium2 kernel.

loss = mean_b clip(||x_b - centers[labels_b]||^2, 1e-12, 1e12)

Shapes (hardcoded): x [8192, 512] f32, labels [8192] int64 in [0, 10000),
centers [10000, 512] f32.  Output: f32 scalar.

Strategy: data-parallel over batch across 8 cores (1024 rows each);
centers stay in HBM (replicated input) and each core gathers exactly the
1024 rows it needs with indirect DMAs (labels as row offsets).  Only the
diagonal entries distmat[b, labels_b] of the reference's [B, C] distmat
are needed, so the kernel is memory-bound.

v5 changes vs the 29.4 us baseline:
  - x and centers are cast to bf16 on the host: halves HBM traffic and
    doubles DVE throughput (~1e-5 rel err against a 2e-2 budget).
  - Each core's rows are sorted by label on the host (the mean is
    order-agnostic) so each 128-row gather reads ascending HBM
    addresses from a ~1/8 slice of centers.
  - Square+reduce is split across engines: tiles 0-5 on ACT (Square
    activation with row-accumulate), tiles 6-7 on DVE (mult + reduce),
    balancing the two pipelines instead of serializing everything
    behind ACT (7.2 us).

The gather HAS to be 8 indirect DMAs of 128 rows: the SWDGE indirect
path takes exactly one row offset per partition (a [128, k] offset AP
silently uses only column 0 and copies k*D contiguous elements), and
each instruction costs ~1 us fixed + ~0.3 us gap on the Q7
descriptor-generation path, so the ~11 us serial gather chain is the
hard floor of this design.

Per-core layout: batch row r = p*8 + t maps to SBUF partition p, tile t
(8 tiles of [128, 512]).  The [128, 8] per-row distances go back to the
host, which applies clip and the global mean.
"""

import sys

import numpy as np

try:
    import concourse  # noqa: F401
except ImportError:  # pragma: no cover
    sys.path.insert(0, "/opt/trn_rl_repo")

import ml_dtypes

B, D, C = 8192, 512, 10000
N_CORES = 8
P = 128
ROWS = B // N_CORES  # 1024 rows per core
T = ROWS // P        # 8 tiles of 128 rows
ACT_TILES = 6        # tiles 0..5 reduce on ACT, the rest on DVE

CLAMP_MIN = 1e-12
CLAMP_MAX = 1e12

_CACHE = {}


def _build():
    import concourse.bacc as bacc
    import concourse.tile as tile
    from concourse import bass, mybir

    f32 = mybir.dt.float32
    bf16 = mybir.dt.bfloat16
    i32 = mybir.dt.int32

    nc = bacc.Bacc("TRN2", target_bir_lowering=False, num_devices=N_CORES)
    x = nc.dram_tensor("x", [ROWS, D], bf16, kind="ExternalInput")
    labels = nc.dram_tensor("labels", [ROWS, 1], i32, kind="ExternalInput")
    centers = nc.dram_tensor("centers", [C, D], bf16, kind="ExternalInput")
    out = nc.dram_tensor("out", [P, T], f32, kind="ExternalOutput")

    with tile.TileContext(nc) as tc:
        with (
            tc.tile_pool(name="big", bufs=1) as big,
            tc.tile_pool(name="small", bufs=1) as small,
            tc.tile_pool(name="work", bufs=4) as work,
        ):
            idx = small.tile([P, T], i32)
            dist = small.tile([P, T], f32)
            xbig = big.tile([P, T * D], bf16)
            cbig = big.tile([P, T * D], bf16)

            # idx[p, t] = labels[p*T + t]; 32 B contiguous per partition.
            nc.sync.dma_start(
                out=idx[:], in_=labels[:, :].rearrange("(p t) o -> p (t o)", p=P)
            )
            # xbig[p, t*D:(t+1)*D] = x[p*T + t, :]; 4 chunks of 2 tiles,
            # alternating between the two HWDGE rings (sync=SP, scalar=ACT).
            xsrc = x[:, :].rearrange("(p t) d -> p (t d)", p=P)
            for k in range(4):
                sl = slice(k * 2 * D, (k + 1) * 2 * D)
                eng = nc.sync if k % 2 == 0 else nc.scalar
                eng.dma_start(out=xbig[:, sl], in_=xsrc[:, sl])

            # cbig[p, t*D:(t+1)*D] = centers[idx[p, t], :].  One indirect
            # DMA per 128 rows (the HW takes one row offset per partition).
            for t in range(T):
                dsl = slice(t * D, (t + 1) * D)
                nc.gpsimd.indirect_dma_start(
                    out=cbig[:, dsl],
                    out_offset=None,
                    in_=centers[:, :],
                    in_offset=bass.IndirectOffsetOnAxis(ap=idx[:, t : t + 1], axis=0),
                )
            for t in range(T):
                sl = slice(t * D, (t + 1) * D)
                diff = work.tile([P, D], bf16, tag="diff")
                nc.vector.tensor_sub(diff[:], xbig[:, sl], cbig[:, sl])
                if t < ACT_TILES:
                    # sq = diff^2 on ACT; dist[:, t] = row-sum via ACT accum.
                    sq = work.tile([P, D], bf16, tag="sq")
                    nc.scalar.activation(
                        sq[:],
                        diff[:],
                        mybir.ActivationFunctionType.Square,
                        accum_out=dist[:, t : t + 1],
                    )
                else:
                    # DVE path: mult + row-reduce, balancing the ACT queue.
                    sq = work.tile([P, D], bf16, tag="sq")
                    nc.vector.tensor_tensor(
                        out=sq[:], in0=diff[:], in1=diff[:], op=mybir.AluOpType.mult
                    )
                    nc.vector.tensor_reduce(
                        out=dist[:, t : t + 1],
                        in_=sq[:],
                        axis=mybir.AxisListType.X,
                        op=mybir.AluOpType.add,
                    )
            nc.sync.dma_start(out=out[:, :], in_=dist[:])

    nc.compile()
    return nc


def get_nc():
    nc = _CACHE.get("nc")
    if nc is None:
        nc = _CACHE["nc"] = _build()
    return nc


def make_in_maps(x, labels, centers):
    labels_i32 = np.asarray(labels).astype(np.int32)
    x16 = np.asarray(x).astype(ml_dtypes.bfloat16)
    c16 = np.ascontiguousarray(np.asarray(centers).astype(ml_dtypes.bfloat16))
    # Slot (p, t) holds shard row p*T+t; remap so tile t gets the 128
    # label-sorted rows [t*128, (t+1)*128) - each gather instruction then
    # reads ascending addresses from a ~1/8 window of centers.  The loss
    # is a mean, so the row order never needs to be undone.
    j = np.arange(ROWS)
    slot_to_sorted = (j % T) * P + j // T
    in_maps = []
    for i in range(N_CORES):
        lo, hi = i * ROWS, (i + 1) * ROWS
        lab = labels_i32[lo:hi]
        ord_ = np.argsort(lab, kind="stable")[slot_to_sorted]
        in_maps.append(
            {
                "x": np.ascontiguousarray(x16[lo:hi][ord_]),
                "labels": np.ascontiguousarray(lab[ord_].reshape(ROWS, 1)),
                "centers": c16,
            }
        )
    return in_maps


def finish(per_core_outs):
    """per_core_outs: list of 8 [P, T] arrays -> f32 scalar loss."""
    d = np.concatenate([np.asarray(o).reshape(-1) for o in per_core_outs])
    d = np.clip(d, CLAMP_MIN, CLAMP_MAX)
    return np.asarray(np.mean(d, dtype=np.float64), dtype=np.float32)


def kernel(x, labels, centers):
    from concourse.bass_utils import run_bass_kernel_spmd

    nc = get_nc()
    in_maps = make_in_maps(x, labels, centers)
    res = run_bass_kernel_spmd(nc, in_maps, core_ids=list(range(N_CORES)))
    return finish([r["out"] for r in res.results])


# revision 9
# speedup vs baseline: 1.0853x; 1.0853x over previous
"""CenterLoss Trainium2 kernel.

loss = mean_b clip(||x_b - centers[labels_b]||^2, 1e-12, 1e12)

Shapes (hardcoded): x [8192, 512] f32, labels [8192] int64 in [0, 10000),
centers [10000, 512] f32.  Output: f32 scalar.

Strategy: data-parallel over batch across 8 cores (1024 rows each);
centers stay in HBM (replicated input) and each core gathers exactly the
1024 rows it needs with indirect DMAs (labels as row offsets).  Only the
diagonal entries distmat[b, labels_b] of the reference's [B, C] distmat
are needed, so the kernel never forms the matmul.

vs the 29.4 us tile-framework baseline:
  - x and centers are cast to bf16 on the host: halves HBM traffic and
    doubles DVE throughput (~1e-5 rel err against a 2e-2 budget).
  - Raw engine programs (no TileContext): the sync engine issues the
    labels DMA ~0.5 us after its preamble instead of behind the tile
    start barrier, the 8 SWDGE gathers run back-to-back on gpsimd, and
    the tile end-of-context drain/barrier/semaphore-clear is gone.
  - Square+reduce is split across engines: tiles 2-7 on ACT (Square
    activation with row-accumulate), tiles 0-1 on DVE (mult + reduce)
    slotted into the slack while DVE waits for later gathers.
  - Each core's rows are sorted by label on the host (the mean is
    order-agnostic) so each 128-row gather reads ascending HBM
    addresses from a ~1/8 window of centers.

The gather is the hard floor of the design: the SWDGE indirect path
takes one row offset per partition (max 128 rows per instruction; a
[128, k] offset AP silently uses only column 0) and each instruction
costs ~1 us fixed (SWDGE_FIXED_OVERHEAD) + ~0.3 us dispatch gap of
serial Q7 time, so the 8-instruction chain is ~11 us regardless of
dtype or queue count (descriptor generation is single-threaded; extra
SWDGE queues don't parallelize it).

Per-core layout: batch row r = p*8 + t maps to SBUF partition p, tile t
(8 tiles of [128, 512]).  The [128, 8] per-row distances go back to the
host, which applies clip and the global mean (f64 accumulate).
"""

import sys

import numpy as np

try:
    import concourse  # noqa: F401
except ImportError:  # pragma: no cover
    sys.path.insert(0, "/opt/trn_rl_repo")

import ml_dtypes

B, D, C = 8192, 512, 10000
N_CORES = 8
P = 128
ROWS = B // N_CORES  # 1024 rows per core
T = ROWS // P        # 8 tiles of 128 rows
DVE_TILES = (0, 1)   # square+reduce on DVE, rest on ACT

XCHUNKS = 2
TPX = T // XCHUNKS   # tiles per x chunk

CLAMP_MIN = 1e-12
CLAMP_MAX = 1e12

_CACHE = {}


def _build():
    import concourse.bacc as bacc
    from concourse import bass, mybir

    f32 = mybir.dt.float32
    bf16 = mybir.dt.bfloat16
    i32 = mybir.dt.int32

    nc = bacc.Bacc("TRN2", target_bir_lowering=False, num_devices=N_CORES)
    x = nc.dram_tensor("x", [ROWS, D], bf16, kind="ExternalInput")
    labels = nc.dram_tensor("labels", [ROWS, 1], i32, kind="ExternalInput")
    centers = nc.dram_tensor("centers", [C, D], bf16, kind="ExternalInput")
    out = nc.dram_tensor("out", [P, T], f32, kind="ExternalOutput")

    from contextlib import ExitStack

    with (
        nc.Block() as block,
        nc.sbuf_tensor("idx", [P, T], i32) as idx,
        nc.sbuf_tensor("xbig", [P, T * D], bf16) as xbig,
        nc.sbuf_tensor("cbig", [P, T * D], bf16) as cbig,
        nc.sbuf_tensor("diffb", [P, T * D], bf16) as diffb,
        nc.sbuf_tensor("sqb", [P, T * D], bf16) as sqb,
        nc.sbuf_tensor("dist", [P, T], f32) as dist,
        nc.semaphore("s_idx") as s_idx,
        nc.semaphore("s_sub") as s_sub,
        nc.semaphore("s_m") as s_m,
        nc.semaphore("s_dist") as s_dist,
        nc.semaphore("s_out") as s_out,
        ExitStack() as stack,
    ):
        # One semaphore per DMA: a shared counter is unsound (HWDGE fans a
        # wide transfer across queues, so increments from different DMAs
        # interleave - partial thresholds guarantee nothing).
        s_x = [stack.enter_context(nc.semaphore(f"s_x{k}")) for k in range(XCHUNKS)]  # noqa: ANT232
        s_g = [stack.enter_context(nc.semaphore(f"s_g{t}")) for t in range(T)]  # noqa: ANT232

        xsrc = x[:, :].rearrange("(p t) d -> p (t d)", p=P)
        lsrc = labels[:, :].rearrange("(p t) o -> p (t o)", p=P)

        @block.sync
        def _(sp):
            sp.dma_start(idx[:, :], lsrc).then_inc(s_idx, 16)
            for k in range(XCHUNKS):
                sl = slice(k * TPX * D, (k + 1) * TPX * D)
                sp.dma_start(xbig[:, sl], xsrc[:, sl]).then_inc(s_x[k], 16)
            sp.wait_ge(s_dist, T)
            sp.dma_start(out[:, :], dist[:, :]).then_inc(s_out, 16)
            sp.wait_ge(s_out, 16)

        @block.gpsimd
        def _(gp):
            gp.wait_ge(s_idx, 16)
            for t in range(T):
                gp.indirect_dma_start(
                    out=cbig[:, t * D : (t + 1) * D],
                    out_offset=None,
                    in_=centers[:, :],
                    in_offset=bass.IndirectOffsetOnAxis(ap=idx[:, t : t + 1], axis=0),
                ).then_inc(s_g[t], 16)

        @block.vector
        def _(v):
            # A DVE op that reads an earlier DVE op's output waits on the
            # producer's tick: back-to-back DVE instructions pipeline, so
            # engine order alone does not order the memory accesses.
            def sub(t):
                v.wait_ge(s_g[t], 16)
                v.wait_ge(s_x[t // TPX], 16)
                sl = slice(t * D, (t + 1) * D)
                v.tensor_sub(diffb[:, sl], xbig[:, sl], cbig[:, sl]).then_inc(
                    s_sub, 1
                )

            mults = []

            def sq_red(t):
                sl = slice(t * D, (t + 1) * D)
                v.wait_ge(s_sub, t + 1)  # diffb[t] written
                v.tensor_tensor(
                    out=sqb[:, sl],
                    in0=diffb[:, sl],
                    in1=diffb[:, sl],
                    op=mybir.AluOpType.mult,
                ).then_inc(s_m, 1)
                v.wait_ge(s_m, len(mults) + 1)  # sqb[t] written
                mults.append(t)
                v.tensor_reduce(
                    out=dist[:, t : t + 1],
                    in_=sqb[:, sl].rearrange("p (o d) -> p o d", o=1),
                    axis=mybir.AxisListType.X,
                    op=mybir.AluOpType.add,
                ).then_inc(s_dist, 1)

            # subs in tile order; DVE-path square+reduce for tiles 0, 1
            # slotted into the slack while waiting for later gathers.
            sub(0)
            sub(1)
            sub(2)
            sq_red(0)
            sub(3)
            sq_red(1)
            for t in range(4, T):
                sub(t)

        @block.scalar
        def _(act):
            for t in range(T):
                if t in DVE_TILES:
                    continue
                act.wait_ge(s_sub, t + 1)
                sl = slice(t * D, (t + 1) * D)
                act.activation(
                    sqb[:, sl],
                    diffb[:, sl],
                    mybir.ActivationFunctionType.Square,
                    accum_out=dist[:, t : t + 1],
                ).then_inc(s_dist, 1)

    nc.compile()
    return nc


def get_nc():
    nc = _CACHE.get("nc")
    if nc is None:
        nc = _CACHE["nc"] = _build()
    return nc


def make_in_maps(x, labels, centers):
    labels_i32 = np.asarray(labels).astype(np.int32)
    x16 = np.asarray(x).astype(ml_dtypes.bfloat16)
    c16 = np.ascontiguousarray(np.asarray(centers).astype(ml_dtypes.bfloat16))
    # Slot (p, t) holds shard row p*T+t; remap so tile t gets the 128
    # label-sorted rows [t*128, (t+1)*128) - each gather instruction then
    # reads ascending addresses from a ~1/8 window of centers.  The loss
    # is a mean, so the row order never needs to be undone.
    j = np.arange(ROWS)
    slot_to_sorted = (j % T) * P + j // T
    in_maps = []
    for i in range(N_CORES):
        lo, hi = i * ROWS, (i + 1) * ROWS
        lab = labels_i32[lo:hi]
        ord_ = np.argsort(lab, kind="stable")[slot_to_sorted]
        in_maps.append(
            {
                "x": np.ascontiguousarray(x16[lo:hi][ord_]),
                "labels": np.ascontiguousarray(lab[ord_].reshape(ROWS, 1)),
                "centers": c16,
            }
        )
    return in_maps


def finish(per_core_outs):
    """per_core_outs: list of 8 [P, T] arrays -> f32 scalar loss."""
    d = np.concatenate([np.asarray(o).reshape(-1) for o in per_core_outs])
    d = np.clip(d, CLAMP_MIN, CLAMP_MAX)
    return np.asarray(np.mean(d, dtype=np.float64), dtype=np.float32)


def kernel(x, labels, centers):
    from concourse.bass_utils import run_bass_kernel_spmd

    nc = get_nc()
    in_maps = make_in_maps(x, labels, centers)
    res = run_bass_kernel_spmd(nc, in_maps, core_ids=list(range(N_CORES)))
    return finish([r["out"] for r in res.results])
